# revision 9
# baseline (speedup 1.0000x reference)
"""HRALinear forward on 8 Trainium2 NeuronCores (Bass/Tile).

Math (compact-WY form of the sequential Householder scan):
  u_i = hra_u[:, i] / ||hra_u[:, i]||
  H_0 H_1 ... H_{r-1} = I - U T U^T          (T upper triangular, T_ii = 2)
  out = X W^T - (X u) T^T (W u)^T + bias
      = X W^T + (X Uraw) S' (W Uraw)^T + 1 x bias^T
  with S' = -D T^T D, D = diag(1/||u_i||)    (S' is 8x8, host-computed)

Sharding: data-parallel over the 8192 batch*seq rows (1024 rows/core);
base_weight / hra_u / bias replicated.  Inputs are uploaded pre-transposed
in a partition-split layout so every device DMA is a natural
(contiguous-per-partition) load; all heavy compute (X W^T, X U, W U and the
rank-8 correction) runs on the PE array in float32r.

Device layout (per core, out^T form):
  psum[o_tile 128, m_blk 512] = sum_kk wpanel[kk,o_tile].T @ xt[kk, m_blk]
                              + at[o_tile].T(S'-folded) @ qones[m_blk]
  eviction via ScalarE activation(Copy, bias=bias[o]) adds bias per partition.
"""

import os
import sys
from contextlib import ExitStack

os.environ.setdefault("MYCRO_LOCAL_CACHE", "1")
for _p in ("/opt/trn_rl_repo",):
    if os.path.isdir(_p) and _p not in sys.path:
        sys.path.insert(0, _p)

import numpy as np

import concourse.bacc as bacc
import concourse.mybir as mybir
import concourse.tile as tile
from concourse.bass_utils import run_bass_kernel_spmd

P = 128          # partitions
N_CORES = 8

F32 = mybir.dt.float32
F32R = mybir.dt.float32r


def build_nc(M, N, K, R):
    """One-core SPMD program: outT[N,M] = wT.T-accumulated x-shard product.

    DRAM inputs (per core):
      xt    [P, K/P, M]  x-shard^T, d split partition-major (d = kk*P + p)
      wt    [P, K/P, N]  W^T, same d split (replicated)
      ut    [P, K/P, R]  hra_u, same d split
      sneg  [R, R]       S' = -D T^T D
      bias2 [P, N/P]     bias2[p, ot] = bias[ot*P + p]
    DRAM output: outT [N/P, P, M]   (outT[ot, p, m] = out[m, ot*P+p])
    """
    KK = K // P
    NT = N // P
    MBW = min(512, M)
    MB = M // MBW
    MH = min(512, M)
    PH = M // MH

    nc = bacc.Bacc()
    xt = nc.dram_tensor("xt", [P, KK, M], F32R, kind="ExternalInput")
    wt = nc.dram_tensor("wt", [P, KK, N], F32R, kind="ExternalInput")
    ut = nc.dram_tensor("ut", [P, KK, R], F32R, kind="ExternalInput")
    sneg = nc.dram_tensor("sneg", [R, R], F32R, kind="ExternalInput")
    bias2 = nc.dram_tensor("bias2", [P, NT], F32, kind="ExternalInput")
    outd = nc.dram_tensor("out", [NT, P, M], F32, kind="ExternalOutput")

    with tile.TileContext(nc) as tc, ExitStack() as ctx:
        const = ctx.enter_context(tc.tile_pool(name="const", bufs=1))
        xpool = ctx.enter_context(tc.tile_pool(name="xpool", bufs=1))
        wpool = ctx.enter_context(tc.tile_pool(name="wpool", bufs=2))
        stage = ctx.enter_context(tc.tile_pool(name="stage", bufs=4))
        at_pool = ctx.enter_context(tc.tile_pool(name="atp", bufs=3))
        pq_pool = ctx.enter_context(tc.tile_pool(name="pq", bufs=1))
        ps_out = ctx.enter_context(tc.tile_pool(name="ps_out", bufs=4, space="PSUM"))
        ps_pq = ctx.enter_context(tc.tile_pool(name="ps_pq", bufs=2, space="PSUM"))
        ps_a = ctx.enter_context(tc.tile_pool(name="ps_a", bufs=2, space="PSUM"))

        s_sb = const.tile([R, R], F32R)
        nc.sync.dma_start(out=s_sb[:], in_=sneg[:])
        u_sb = const.tile([P, KK * R], F32R)
        nc.sync.dma_start(out=u_sb[:], in_=ut[:, :, :])
        bias_sb = const.tile([P, NT], F32)
        nc.sync.dma_start(out=bias_sb[:], in_=bias2[:])

        qones = pq_pool.tile([R, M], F32R, tag="qones")
        praw = pq_pool.tile([R, M], F32R, tag="praw")

        xt_sb = xpool.tile([P, KK * M], F32R)
        for kk in range(KK):
            nc.sync.dma_start(out=xt_sb[:, kk * M : (kk + 1) * M], in_=xt[:, kk, :])

        for ot in range(NT):
            o0 = ot * P
            wpanel = wpool.tile([P, KK * P], F32R, tag="wpanel")
            nc.sync.dma_start(out=wpanel[:, :], in_=wt[:, :, o0 : o0 + P])

            # A^T[:, o-slice] = sum_kk U_k^T @ Wpanel_k   -> [R, P]
            psa = ps_a.tile([R, P], F32, tag="ps_a")
            for kk in range(KK):
                nc.tensor.matmul(
                    psa[:],
                    u_sb[:, kk * R : (kk + 1) * R],
                    wpanel[:, kk * P : (kk + 1) * P],
                    start=(kk == 0),
                    stop=(kk == KK - 1),
                )
            at = at_pool.tile([R, P], F32R, tag="at")
            nc.vector.tensor_copy(at[:], psa[:])

            # main: psum[o 128, m 512] accumulated over kk
            psos = []
            for mb in range(MB):
                pso = ps_out.tile([P, MBW], F32, tag="ps_out", name=f"pso{ot}_{mb}")
                psos.append(pso)
                for kk in range(KK):
                    nc.tensor.matmul(
                        pso[:],
                        wpanel[:, kk * P : (kk + 1) * P],
                        xt_sb[:, kk * M + mb * MBW : kk * M + (mb + 1) * MBW],
                        start=(kk == 0),
                        stop=False,
                    )

            if ot == 0:
                # P^T = (x u)^T rides the xt residency -> [R, M]
                for h in range(PH):
                    ppq = ps_pq.tile([R, MH], F32, tag="ps_pq", name=f"ppq{h}")
                    for kk in range(KK):
                        nc.tensor.matmul(
                            ppq[:],
                            u_sb[:, kk * R : (kk + 1) * R],
                            xt_sb[:, kk * M + h * MH : kk * M + (h + 1) * MH],
                            start=(kk == 0),
                            stop=(kk == KK - 1),
                        )
                    nc.vector.tensor_copy(praw[:, h * MH : (h + 1) * MH], ppq[:])
                for h in range(PH):
                    q_t = ps_pq.tile([R, MH], F32, tag="ps_pq", name=f"q_t{h}")
                    nc.tensor.matmul(
                        q_t[:],
                        s_sb[:],
                        praw[:, h * MH : (h + 1) * MH],
                        start=True,
                        stop=True,
                    )
                    nc.vector.tensor_copy(qones[:, h * MH : (h + 1) * MH], q_t[:])

            for mb in range(MB):
                # rank-R correction accumulated into the same PSUM group
                nc.tensor.matmul(
                    psos[mb][:],
                    at[:],
                    qones[:, mb * MBW : (mb + 1) * MBW],
                    start=False,
                    stop=True,
                )
                st = stage.tile([P, MBW], F32, tag="stage")
                # eviction on ScalarE with per-partition bias add
                nc.scalar.activation(
                    st[:],
                    psos[mb][:],
                    mybir.ActivationFunctionType.Identity,
                    bias=bias_sb[:, ot : ot + 1],
                )
                nc.sync.dma_start(
                    out=outd[ot, :, mb * MBW : (mb + 1) * MBW], in_=st[:]
                )

    nc.compile()
    return nc


_NC_CACHE = {}


def get_nc(M, N, K, R):
    key = (M, N, K, R)
    if key not in _NC_CACHE:
        _NC_CACHE[key] = build_nc(M, N, K, R)
    return _NC_CACHE[key]


def compute_sneg(hra_u):
    R = hra_u.shape[1]
    U = np.asarray(hra_u, dtype=np.float64)
    nrm = np.linalg.norm(U, axis=0)
    Uh = U / nrm
    G = Uh.T @ Uh
    T = np.zeros((R, R))
    for k in range(R):
        T[k, k] = 2.0
        if k:
            T[:k, k] = -2.0 * (T[:k, :k] @ G[:k, k])
    return (-(T.T) / nrm[:, None] / nrm[None, :]).astype(np.float32)


def part_split(a, _unused=None):
    """[K, F] row-major -> [P, K/P, F] with K = kk*P + p."""
    K, F = a.shape
    return np.ascontiguousarray(a.reshape(K // P, P, F).transpose(1, 0, 2))


def prepare(x, hra_u, base_weight, bias):
    x = np.asarray(x, dtype=np.float32)
    hra_u = np.asarray(hra_u, dtype=np.float32)
    base_weight = np.asarray(base_weight, dtype=np.float32)
    bias = np.asarray(bias, dtype=np.float32)

    B, S, K = x.shape
    N = base_weight.shape[0]
    R = hra_u.shape[1]
    Mtot = B * S
    M = Mtot // N_CORES

    X = x.reshape(Mtot, K)
    wtp = part_split(np.ascontiguousarray(base_weight.T))  # [P, K/P, N]
    utp = part_split(hra_u)                                # [P, K/P, R]
    sneg = compute_sneg(hra_u)
    bias2 = np.ascontiguousarray(bias.reshape(N // P, P).T)  # [P, N/P]

    nc = get_nc(M, N, K, R)

    in_maps = []
    for c in range(N_CORES):
        shard = X[c * M : (c + 1) * M]
        xtp = part_split(np.ascontiguousarray(shard.T))    # [P, K/P, M]
        in_maps.append(
            {"xt": xtp, "wt": wtp, "ut": utp, "sneg": sneg, "bias2": bias2}
        )
    return nc, in_maps, (B, S, M, N)


def collect(res, meta):
    B, S, M, N = meta
    shards = [r["out"].reshape(N, M).T for r in res]       # outT -> [M, N]
    out = np.concatenate(shards, axis=0)
    return np.ascontiguousarray(out.reshape(B, S, N), dtype=np.float32)


def kernel(x, hra_u, base_weight, bias):
    nc, in_maps, meta = prepare(x, hra_u, base_weight, bias)
    res = run_bass_kernel_spmd(nc, in_maps, core_ids=list(range(N_CORES))).results
    return collect(res, meta)
